# revision 5
# baseline (speedup 1.0000x reference)
"""Trainium2 Bass kernel for nn_Coo2Cel (periodic pairwise displacement grid).

Reference semantics (B=1, N=1024 atoms, diagonal 30 A cell, rc=6):
  out[b,i,j,s,:] = (vec, sod), vec = pos_i - pos_j - shift_s, sod = |vec|^2,
  zeroed unless sod < rc^2 (self-pair at zero shift also zeroed).

Key structure: box=30 > 2*rc=12, so for every (i,j) pair AT MOST ONE of the
27 shifts can pass the cutoff -- the minimum-image shift with
sigma_c = round(d_c/box) for d = pos_i - pos_j, d in (-30, 30).  Proof: if
|d - 30*sigma| <= 15 then any other sigma' has |d - 30*sigma'| >= 15 > rc.

So the device computes, for all (i,j) pairs, only the minimum-image data:
    n_c  = (d_c >= 15) - (d_c <= -15)        (exact, in {-1,0,1})
    w_c  = d_c - 30*n_c                      (bit-exact vs reference f32)
    sod  = (wx^2 + wy^2) + wz^2              (same eval order as reference)
    sneg = 9*mx + 3*my + mz  where m = -n    (shift index code, exact ints)
and emits five [P, N] f32 planes per core (2.6 MB vs 56.6 MB dense).  The
host allocates the zero-filled dense tensor and scatters the (~34 per row)
entries with sod < rc^2 to out[i, j, 13 - sneg, :] -- pure format
conversion; every nonzero value is device-computed.

Sharding: query rows i split row-wise across 8 cores (128 rows = 128 SBUF
partitions per core); every core holds all N candidates.  No collectives.
"""
import sys

if "/opt/trn_rl_repo" not in sys.path:
    sys.path.insert(0, "/opt/trn_rl_repo")

import numpy as np

N = 1024          # atoms
S = 27            # lattice shifts
P = 128           # partitions / query rows per core
NCORES = 8
RC2 = 36.0        # rc^2, rc = 6.0
JT = 256          # candidate tile size
NT = N // JT

TRACE = False          # set by test harness to collect a profile
LAST_RESULT = None     # BassKernelResults of the last run (for profiling)

_CACHE = {}


def _build(box, pbc_tuple, repeat=1):
    import concourse.bacc as bacc
    import concourse.mybir as mybir
    from concourse.tile import TileContext

    F32 = mybir.dt.float32
    ADD = mybir.AluOpType.add
    MULT = mybir.AluOpType.mult
    SUB = mybir.AluOpType.subtract
    ISGE = mybir.AluOpType.is_ge
    ISLE = mybir.AluOpType.is_le
    half = tuple(float(b) * 0.5 for b in box)

    nc = bacc.Bacc()
    qin_d = nc.declare_dram_parameter("qin", [P, 3, 1], F32, isOutput=False)
    pin_d = nc.declare_dram_parameter("pin", [P, 3, N], F32, isOutput=False)
    out_d = nc.declare_dram_parameter("out", [P, 5, N], F32, isOutput=True)

    with TileContext(nc) as tc:
        with (
            tc.tile_pool(name="const", bufs=1) as cpool,
            tc.tile_pool(name="work", bufs=2) as wpool,
            tc.tile_pool(name="outp", bufs=3) as opool,
        ):
            qin = cpool.tile([P, 3, 1], F32)
            pin = cpool.tile([P, 3, N], F32)
            # query column on the HWDGE path, candidate chunks on the
            # SWDGE (gpsimd) path: independent queues, both ahead of compute
            nc.sync.dma_start(out=qin[:], in_=qin_d[:])
            for ct in range(NT):
                cs = slice(ct * JT, (ct + 1) * JT)
                nc.gpsimd.dma_start(out=pin[:, :, cs], in_=pin_d[:, :, cs])

            for rep in range(repeat):
                for jt in range(NT):
                    js = slice(jt * JT, (jt + 1) * JT)
                    d = wpool.tile([P, 3, JT], F32, tag="d")
                    a = wpool.tile([P, 3, JT], F32, tag="a")
                    m = wpool.tile([P, 3, JT], F32, tag="m")
                    sq = wpool.tile([P, 3, JT], F32, tag="sq")
                    t1 = wpool.tile([P, JT], F32, tag="t1")
                    t2 = wpool.tile([P, JT], F32, tag="t2")
                    outt = opool.tile([P, 5, JT], F32, tag="outt")
                    # d = q - p   (query column broadcast along j)
                    nc.vector.tensor_tensor(
                        out=d[:],
                        in0=qin[:].broadcast_to([P, 3, JT]),
                        in1=pin[:, :, js],
                        op=SUB,
                    )
                    # a = (d >= box/2)
                    nc.vector.tensor_scalar(
                        out=a[:], in0=d[:], scalar1=half[0], scalar2=None,
                        op0=ISGE)
                    # m = (d <= -box/2) - a   == -n
                    nc.vector.scalar_tensor_tensor(
                        out=m[:], in0=d[:], scalar=-half[0], in1=a[:],
                        op0=ISLE, op1=SUB)
                    for c in range(3):
                        if not pbc_tuple[c]:
                            nc.vector.memset(m[:, c, :], 0.0)
                    # w = box*m + d   (minimum image, exact)
                    nc.vector.scalar_tensor_tensor(
                        out=outt[:, 0:3, :], in0=m[:], scalar=float(box[0]),
                        in1=d[:], op0=MULT, op1=ADD)
                    # squares on ScalarE (own SBUF port)
                    nc.scalar.activation(
                        out=sq[:], in_=outt[:, 0:3, :],
                        func=mybir.ActivationFunctionType.Square)
                    # sod = (sqx + sqy) + sqz  -- reference eval order
                    nc.vector.tensor_tensor(
                        out=t1[:], in0=sq[:, 0, :], in1=sq[:, 1, :], op=ADD)
                    nc.vector.tensor_tensor(
                        out=outt[:, 3, :], in0=t1[:], in1=sq[:, 2, :], op=ADD)
                    # sneg = 9*mx + (3*my + mz)
                    nc.vector.scalar_tensor_tensor(
                        out=t2[:], in0=m[:, 1, :], scalar=3.0, in1=m[:, 2, :],
                        op0=MULT, op1=ADD)
                    nc.vector.scalar_tensor_tensor(
                        out=outt[:, 4, :], in0=m[:, 0, :], scalar=9.0,
                        in1=t2[:], op0=MULT, op1=ADD)
                    nc.sync.dma_start(out=out_d[:, :, js], in_=outt[:])
    nc.finalize()
    return nc


def _prepare(pos_cel, cel_mat, pbc):
    """Host-side shard prep: returns (box, pbc_tuple, in_maps)."""
    pos_cel = np.asarray(pos_cel)
    cel_mat = np.asarray(cel_mat, dtype=np.float32)
    pbc = np.asarray(pbc)
    B = pos_cel.shape[0]
    assert pos_cel.shape == (B, N, 3), pos_cel.shape
    assert B == 1

    # Cartesian positions; exact for the diagonal cell (pos_d = pos_cel_d * box_d)
    pos = (pos_cel[0].astype(np.float32) @ cel_mat[0]).astype(np.float32)
    off = cel_mat[0] - np.diag(np.diag(cel_mat[0]))
    assert np.all(off == 0), "kernel assumes a diagonal cell matrix"
    box = tuple(float(cel_mat[0][c, c]) for c in range(3))
    assert box[0] == box[1] == box[2], "kernel assumes a cubic cell"
    pbc_tuple = tuple(bool(x) for x in pbc[0])

    pT = np.ascontiguousarray(pos.T)          # [3, N]
    in_maps = []
    for k in range(NCORES):
        pin = np.broadcast_to(pT[None], (P, 3, N))
        pin = np.ascontiguousarray(pin)
        qin = np.ascontiguousarray(pos[k * P:(k + 1) * P])[:, :, None]  # [P,3,1]
        in_maps.append({"pin": pin, "qin": qin})
    return box, pbc_tuple, in_maps


def kernel(pos_cel, cel_mat, pbc):
    global LAST_RESULT
    from concourse.bass_utils import run_bass_kernel_spmd

    box, pbc_tuple, in_maps = _prepare(pos_cel, cel_mat, pbc)
    key = (box, pbc_tuple)
    if key not in _CACHE:
        _CACHE[key] = _build(box, pbc_tuple)
    nc = _CACHE[key]

    res = run_bass_kernel_spmd(nc, in_maps, list(range(NCORES)), trace=TRACE)
    LAST_RESULT = res

    out = np.zeros((1, N, N, S, 4), dtype=np.float32)
    flat = out.reshape(N * N * S, 4)
    for k in range(NCORES):
        arr = np.asarray(res.results[k]["out"]).reshape(P, 5, N)
        sod = arr[:, 3]
        ii, jj = np.nonzero(sod < RC2)
        sidx = 13 - arr[:, 4][ii, jj].astype(np.int64)
        gi = k * P + ii
        idx = (gi * N + jj) * S + sidx
        flat[idx, 0] = arr[:, 0][ii, jj]
        flat[idx, 1] = arr[:, 1][ii, jj]
        flat[idx, 2] = arr[:, 2][ii, jj]
        flat[idx, 3] = sod[ii, jj]
    return out
